# revision 36
# baseline (speedup 1.0000x reference)
"""Trainium2 Bass kernel: 2-layer GRU (H=128) over 28 timesteps + Linear head.

Reference computation (PyTorch GRUCell semantics, gates r,z,n):
    for t in 28 rows of each 28x28 image:
        h1 = relu(gru1(x_t, h1));  h2 = relu(gru2(h1, h2))
    out = h2 @ w_out.T + b_out

Sharding: pure data parallel, batch 32768 -> 8 cores x 4096.
On-chip layout: transposed [hidden=partition, batch=free]; batch 8x512,
SBUF elementwise ops pair-wide (1024 cols).

Matmul plan per cell (was 6 bf16 matmuls in the baseline, now 4):
  - r and z gates: ONE fp8 DoubleRow matmul each, fusing the input-side and
    hidden-side GEMMs (virtual K=256: plane0 = x/h1, plane1 = h_prev).
  - n gate: ghn (bf16 weights, fp8 h rhs), plus gin emitted one pipeline slot
    later with start=False, ACCUMULATING onto a psum bank pre-seeded by the
    t1 = (ghn+b)*r vector op -- so the former t2 = t1+gin add is done by the
    PE for free and tanh reads the psum directly.
fp8 h state copies are relu ops writing float8_e4m3 on DVE; the bf16 h state
is eliminated: the recurrence keeps pre-relu hp in bf16 and fuses relu into
the (h - n) subtract via scalar_tensor_tensor(max, subtract).
Validated end-to-end in numpy: rel_err ~7.6e-3 (vs 2e-2 budget).

Engine notes (measured): Pool TENSOR_SCALAR is pathological (~14.7us/1024) --
never use; Pool TENSOR_TENSOR is 2.3us/1024 (0.42 eff) -- used for the hp add
and half the t4 muls. DVE stt has no 2x mode (1.34us/1024).

Bias folding:
  - L1 r/z: ones row in the fp8 x plane; weight row 28 carries b_ih1+b_hh1.
  - L1 n: ones row in the bf16 x tile; w1n row 28 carries b_ih1n.
  - L2 r/z: ScalarE activation bias APs. L2 n: tanh bias AP (b_ih2n).
  - b_hh*n folded into the (ghn + b) * r fused scalar_tensor_tensor.
"""

import json
import os
from contextlib import ExitStack

import ml_dtypes
import numpy as np

import concourse.bass as bass
import concourse.tile as tile
from concourse import mybir
from concourse.bass_utils import run_bass_kernel_spmd

HID = 128
T = 28
C = 28
KAUG = C + 1
NCORES = 8
N_TOTAL = 32768
B_CORE = N_TOTAL // NCORES  # 4096
BF = 512                    # matmul free dim / psum bank
PW = 2 * BF                 # pair width for SBUF elementwise ops
NPAIR = B_CORE // PW        # 4
NOUT = 10

F32 = mybir.dt.float32
BF16 = mybir.dt.bfloat16
FP8 = mybir.dt.float8e4
AF = mybir.ActivationFunctionType
ALU = mybir.AluOpType
DR = mybir.MatmulPerfMode.DoubleRow

NP_FP8 = ml_dtypes.float8_e4m3
NP_BF16 = ml_dtypes.bfloat16

# stash of the last run's perf results for test harness inspection
LAST_RESULT = None


def _split_multi_waits(bir_bytes: bytes) -> bytes:
    """This walrus build rejects instructions carrying >1 sync wait
    ("Too many sync wait commands"). Split extras into standalone
    single-wait EventSemaphore instructions on the same engine, placed
    immediately before -- semantically identical blocking."""
    d = json.loads(bir_bytes)
    ctr = 0
    for fn in d["functions"]:
        for bb in fn["blocks"]:
            out = []
            for inst in bb["instructions"]:
                si = inst.get("sync_info")
                waits = (si or {}).get("on_wait") or []
                if len(waits) > 1:
                    for w in waits[:-1]:
                        ctr += 1
                        out.append({
                            "debug": inst.get("debug", 0),
                            "engine": inst.get("engine"),
                            "ins": [],
                            "outs": [],
                            "name": f"xw-{ctr}",
                            "opcode": "EventSemaphore",
                            "sync_info": {"on_update": [], "on_wait": [w]},
                        })
                    si["on_wait"] = [waits[-1]]
                out.append(inst)
            bb["instructions"] = out
    return json.dumps(d).encode()


def _build_bass() -> bass.Bass:
    nc = bass.Bass()

    x8_d = nc.dram_tensor("x8", [T, KAUG, B_CORE], FP8, kind="ExternalInput")
    # bf16 x for the L1 n-gate, pre-packed for 2-way PE row tiling:
    # rows 0-28 = x of even 512-col half, rows 32-60 = x of odd half
    xb_d = nc.dram_tensor("xb", [T, 64, B_CORE // 2], BF16, kind="ExternalInput")
    w1r_d = nc.dram_tensor("w1r", [HID, 2, HID], FP8, kind="ExternalInput")
    w1z_d = nc.dram_tensor("w1z", [HID, 2, HID], FP8, kind="ExternalInput")
    w2r_d = nc.dram_tensor("w2r", [HID, 2, HID], FP8, kind="ExternalInput")
    w2z_d = nc.dram_tensor("w2z", [HID, 2, HID], FP8, kind="ExternalInput")
    w1n_d = nc.dram_tensor("w1n", [64, HID], BF16, kind="ExternalInput")
    whn1_d = nc.dram_tensor("whn1", [HID, HID], BF16, kind="ExternalInput")
    win2_d = nc.dram_tensor("win2", [HID, HID], BF16, kind="ExternalInput")
    whn2_d = nc.dram_tensor("whn2", [HID, HID], BF16, kind="ExternalInput")
    wout_d = nc.dram_tensor("woutT", [HID, NOUT], BF16, kind="ExternalInput")
    # bias columns: 0=b2r, 1=b2z, 2=b_hh1n, 3=b_hh2n, 4=b_ih2n
    bias_d = nc.dram_tensor("biases", [HID, 5], F32, kind="ExternalInput")
    bout_d = nc.dram_tensor("bout", [NOUT, BF], F32, kind="ExternalInput")
    out_d = nc.dram_tensor("out", [NOUT, B_CORE], F32, kind="ExternalOutput")

    with ExitStack() as ctx:
        tc = ctx.enter_context(tile.TileContext(nc))

        consts = ctx.enter_context(tc.tile_pool(name="consts", bufs=1))
        prz = ctx.enter_context(tc.tile_pool(name="prz", bufs=2, space="PSUM"))
        pgi = ctx.enter_context(tc.tile_pool(name="pgi", bufs=2, space="PSUM"))
        pgh = ctx.enter_context(tc.tile_pool(name="pgh", bufs=2, space="PSUM"))
        xbp = ctx.enter_context(tc.tile_pool(name="xbp", bufs=3))
        l1p = ctx.enter_context(tc.tile_pool(name="l1p", bufs=3))
        l2p = ctx.enter_context(tc.tile_pool(name="l2p", bufs=2))
        hpp = ctx.enter_context(tc.tile_pool(name="hpp", bufs=2))
        rzp = ctx.enter_context(tc.tile_pool(name="rzp", bufs=2))
        t1sc = ctx.enter_context(tc.tile_pool(name="t1sc", bufs=1))
        npl = ctx.enter_context(tc.tile_pool(name="npl", bufs=2))
        t3p = ctx.enter_context(tc.tile_pool(name="t3p", bufs=1))
        t4p = ctx.enter_context(tc.tile_pool(name="t4p", bufs=1))
        opool = ctx.enter_context(tc.tile_pool(name="op", bufs=1))

        w1r = consts.tile([HID, 2, HID], FP8)
        nc.sync.dma_start(out=w1r, in_=w1r_d[:, :, :])
        w1z = consts.tile([HID, 2, HID], FP8)
        nc.sync.dma_start(out=w1z, in_=w1z_d[:, :, :])
        w2r = consts.tile([HID, 2, HID], FP8)
        nc.sync.dma_start(out=w2r, in_=w2r_d[:, :, :])
        w2z = consts.tile([HID, 2, HID], FP8)
        nc.sync.dma_start(out=w2z, in_=w2z_d[:, :, :])
        w1n = consts.tile([64, HID], BF16)
        nc.sync.dma_start(out=w1n, in_=w1n_d[:, :])
        whn1 = consts.tile([HID, HID], BF16)
        nc.sync.dma_start(out=whn1, in_=whn1_d[:, :])
        win2 = consts.tile([HID, HID], BF16)
        nc.sync.dma_start(out=win2, in_=win2_d[:, :])
        whn2 = consts.tile([HID, HID], BF16)
        nc.sync.dma_start(out=whn2, in_=whn2_d[:, :])
        wo = consts.tile([HID, NOUT], BF16)
        nc.sync.dma_start(out=wo, in_=wout_d[:, :])
        bs = consts.tile([HID, 5], F32)
        nc.sync.dma_start(out=bs, in_=bias_d[:, :])
        bo = consts.tile([NOUT, BF], F32)
        nc.sync.dma_start(out=bo, in_=bout_d[:, :])

        # --- prologue: state + rhs staging tiles ---
        l1t = {sp: [] for sp in range(NPAIR)}   # fp8 [x_t | h1_{t-1}] per (t, sp)
        xbt = {sp: [] for sp in range(NPAIR)}   # bf16 x_t per (t, sp)
        l2cur = {}                              # fp8 [h1_t | h2_{t-1}] per sp
        hp = {}                                 # bf16 pre-relu state per (layer, sp)

        for sp in range(NPAIR):
            for t0 in range(3):
                tl = l1p.tile([HID, 2, PW], FP8, tag=f"l1_{sp}", name=f"l1_{sp}_{t0}")
                nc.vector.memset(tl, 0.0)
                nc.sync.dma_start(out=tl[0:KAUG, 0, :],
                                  in_=x8_d[t0, :, sp * PW:(sp + 1) * PW])
                l1t[sp].append(tl)
                xt = xbp.tile([64, BF], BF16, tag=f"xb_{sp}", name=f"xb_{sp}_{t0}")
                nc.sync.dma_start(out=xt, in_=xb_d[t0, :, sp * BF:(sp + 1) * BF])
                xbt[sp].append(xt)
            l2tile = l2p.tile([HID, 2, PW], FP8, tag=f"l2_{sp}", name=f"l2i_{sp}")
            nc.gpsimd.memset(l2tile, 0.0)
            l2cur[sp] = l2tile
            h1i = hpp.tile([HID, PW], BF16, tag=f"hp1_{sp}", name=f"hp1i_{sp}")
            nc.gpsimd.memset(h1i, 0.0)
            hp[(1, sp)] = h1i
            h2i = hpp.tile([HID, PW], BF16, tag=f"hp2_{sp}", name=f"hp2i_{sp}")
            nc.gpsimd.memset(h2i, 0.0)
            hp[(2, sp)] = h2i

        pending_dma = []

        def unit(t, layer, sp):
            """One pair (t, layer, sp). PE order: ghn first (no fresh deps,
            and t1 = (ghn+b)*r is on the critical chain), then 4 DR, then the
            accumulating gin matmuls last -- padded by the 6-matmul batch so
            the in-order PE queue rarely stalls on the sigma->t1 chain."""
            if layer == 1:
                l1c = l1t[sp][t]
                rhs_of = lambda po: l1c[:, :, po * BF:(po + 1) * BF]
                ghn_of = lambda po: l1c[:, 1, po * BF:(po + 1) * BF]
                wr, wz, whn = w1r, w1z, whn1
                sig_r = sig_z = 0.0
                bhhn = bs[:, 2:3]
                tanh_bias = 0.0
            else:
                l2c = l2cur[sp]
                rhs_of = lambda po: l2c[:, :, po * BF:(po + 1) * BF]
                ghn_of = lambda po: l2c[:, 1, po * BF:(po + 1) * BF]
                wr, wz, whn = w2r, w2z, whn2
                sig_r, sig_z = bs[:, 0:1], bs[:, 1:2]
                bhhn = bs[:, 3:4]
                tanh_bias = bs[:, 4:5]

            rzt = rzp.tile([HID, 2, PW], BF16, tag=f"rz_{sp}", name=f"rz{layer}_{sp}")
            # ghn first: it has no fresh dependencies and t1 = (ghn+b)*r
            # is on the critical chain to the accumulating gin matmuls
            gi = {}
            gh = {}
            for po in range(2):
                gh[po] = pgh.tile([HID, BF], F32, tag="gh", name=f"gh_{po}")
                nc.tensor.matmul(gh[po], whn, ghn_of(po), start=True, stop=True)
            rz = {}
            for po in range(2):
                rz[po] = prz.tile([HID, 2 * BF], F32, tag="rz", name=f"rz_{po}")
                nc.tensor.matmul(rz[po][:, 0:BF], wr, rhs_of(po), start=True,
                                 stop=True, perf_mode=DR)
                nc.tensor.matmul(rz[po][:, BF:2 * BF], wz, rhs_of(po),
                                 start=True, stop=True, perf_mode=DR)
            for po in range(2):
                # sigma_r first, then the t1 seed (which only needs r), then
                # sigma_z -- t1 is on the critical chain to the gin matmuls.
                # Priority-boosted so the scheduler queues them ahead of the
                # previous unit's tail ops on the ACT/DVE queues.
                with tc.high_priority(offset=60):
                    nc.scalar.activation(rzt[:, 0, po * BF:(po + 1) * BF],
                                         rz[po][:, 0:BF], AF.Sigmoid,
                                         bias=sig_r)
                    # seed the gin psum bank with t1 = (ghn + b_hhn) * r;
                    # the gin matmul then accumulates on top (start=False)
                    gi[po] = pgi.tile([HID, BF], F32, tag="gi", name=f"gi_{po}")
                    nc.vector.scalar_tensor_tensor(
                        gi[po], gh[po], bhhn,
                        rzt[:, 0, po * BF:(po + 1) * BF],
                        op0=ALU.add, op1=ALU.mult)
                nc.scalar.activation(rzt[:, 1, po * BF:(po + 1) * BF],
                                     rz[po][:, BF:2 * BF], AF.Sigmoid,
                                     bias=sig_z)
            # h1 fp8 copies deferred one slot: by now the producing relu is
            # long done, so the issue never head-of-line-blocks the Sync DMA
            # queue (which cost ~10us/copy when issued eagerly).
            while pending_dma:
                dst, src = pending_dma.pop()
                nc.sync.dma_start(out=dst, in_=src)
            if layer == 1:
                # 2-way row tiling: both 512-col halves' gins run concurrently
                # in PE row groups 0 and 32 (K=29 each)
                xt = xbt[sp][t]
                nc.tensor.matmul(gi[0], w1n[0:KAUG, :], xt[0:KAUG, :],
                                 start=False, stop=True, skip_group_check=True,
                                 tile_position=(0, 0))
                nc.tensor.matmul(gi[1], w1n[32:32 + KAUG, :],
                                 xt[32:32 + KAUG, :],
                                 start=False, stop=True, skip_group_check=True,
                                 tile_position=(32, 0))
            else:
                for po in range(2):
                    nc.tensor.matmul(gi[po], win2,
                                     l2cur[sp][:, 0, po * BF:(po + 1) * BF],
                                     start=False, stop=True,
                                     skip_group_check=True)
            nn = npl.tile([HID, PW], BF16, tag=f"n_{sp}", name=f"n{layer}_{sp}")
            for po in range(2):
                nc.scalar.activation(nn[:, po * BF:(po + 1) * BF], gi[po],
                                     AF.Tanh, bias=tanh_bias)

            def tail():
                # State update + relus, deferred one slot so the NEXT unit's
                # t1 seed precedes these ops on the DVE queue -- breaking the
                # t1 -> tail(prev) -> tanh -> gin -> t1 circular queue wait.
                hp_key = (layer, sp)
                # t3 = relu(hp_prev) - n  (fused relu via max-then-subtract)
                t3 = t3p.tile([HID, PW], BF16, tag=f"t3_{sp}", name="t3")
                nc.vector.scalar_tensor_tensor(t3, hp[hp_key], 0.0, nn,
                                               op0=ALU.max, op1=ALU.subtract)
                t4 = t4p.tile([HID, PW], BF16, tag=f"t4_{sp}", name="t4")
                t4_eng = nc.vector if sp % 2 == 0 else nc.gpsimd
                t4_eng.tensor_tensor(t4, rzt[:, 1, :], t3, op=ALU.mult)
                hpn = hpp.tile([HID, PW], BF16, tag=f"hp{layer}_{sp}",
                               name=f"hp{layer}_{sp}_n")
                nc.gpsimd.tensor_tensor(hpn, nn, t4, op=ALU.add)
                hp[hp_key] = hpn

                if layer == 1:
                    # fp8 relu'd h1 -> L2 rhs plane0 (same t)
                    nc.vector.tensor_scalar_max(l2cur[sp][:, 0, :], hpn, 0.0)
                    # copy h1 fp8 -> next L1 rhs plane1 (issue deferred)
                    if t + 1 < T:
                        pending_dma.append((l1t[sp][t + 1][:, 1, :],
                                            l2cur[sp][:, 0, :]))
                    # stage t+3 inputs (prologue covered 0..2)
                    if t + 3 < T:
                        tl = l1p.tile([HID, 2, PW], FP8, tag=f"l1_{sp}",
                                      name=f"l1_{sp}_{t + 3}")
                        nc.sync.dma_start(
                            out=tl[0:KAUG, 0, :],
                            in_=x8_d[t + 3, :, sp * PW:(sp + 1) * PW])
                        l1t[sp].append(tl)
                        xt = xbp.tile([64, BF], BF16, tag=f"xb_{sp}",
                                      name=f"xb_{sp}_{t + 3}")
                        nc.sync.dma_start(
                            out=xt, in_=xb_d[t + 3, :, sp * BF:(sp + 1) * BF])
                        xbt[sp].append(xt)
                else:
                    if t + 1 < T:
                        l2n = l2p.tile([HID, 2, PW], FP8, tag=f"l2_{sp}",
                                       name=f"l2_{sp}_{t + 1}")
                        nc.vector.tensor_scalar_max(l2n[:, 1, :], hpn, 0.0)
                        l2cur[sp] = l2n

            return tail

        for t in range(T):
            for layer in (1, 2):
                for sp in range(NPAIR):
                    unit(t, layer, sp)()

        # ---- head: out = relu(hp2) @ wout + bout ----
        ob = opool.tile([NOUT, B_CORE], F32, tag="ob")
        for sp in range(NPAIR):
            h2f = npl.tile([HID, PW], BF16, tag=f"n_{sp}", name=f"h2f_{sp}")
            nc.vector.tensor_scalar_max(h2f, hp[(2, sp)], 0.0)
            for po in range(2):
                pout = pgh.tile([NOUT, BF], F32, tag="gh")
                nc.tensor.matmul(pout, wo, h2f[:, po * BF:(po + 1) * BF],
                                 start=True, stop=True)
                s = sp * 2 + po
                nc.vector.tensor_tensor(ob[:, s * BF:(s + 1) * BF], pout, bo,
                                        op=ALU.add)
        nc.scalar.dma_start(out=out_d[:, :], in_=ob)

    return nc


def _prep_inputs(x, w_ih1, w_hh1, b_ih1, b_hh1, w_ih2, w_hh2, b_ih2, b_hh2,
                 w_out, b_out):
    """Host-side reshape/transpose/cast + per-core sharding."""
    n = N_TOTAL
    H = HID
    xs = np.asarray(x, np.float32).reshape(n, T, C)       # channel dim is 1
    xt = np.transpose(xs, (1, 2, 0))                      # [T, C, n]
    xaug = np.concatenate(
        [xt, np.ones((T, 1, n), np.float32)], axis=1)     # [T, KAUG, n]

    w_ih1 = np.asarray(w_ih1, np.float32)
    w_hh1 = np.asarray(w_hh1, np.float32)
    b_ih1 = np.asarray(b_ih1, np.float32)
    b_hh1 = np.asarray(b_hh1, np.float32)
    w_ih2 = np.asarray(w_ih2, np.float32)
    w_hh2 = np.asarray(w_hh2, np.float32)
    b_ih2 = np.asarray(b_ih2, np.float32)
    b_hh2 = np.asarray(b_hh2, np.float32)
    w_out = np.asarray(w_out, np.float32)
    b_out = np.asarray(b_out, np.float32)

    def dr_weights(w_ih, w_hh, bias_row, lo):
        """fp8 DoubleRow lhsT [128, 2, 128] for one gate (rows lo:lo+H)."""
        w = np.zeros((HID, 2, HID), np.float32)
        kin = w_ih.shape[1]
        w[0:kin, 0, :] = w_ih[lo:lo + H].T
        if bias_row is not None:
            w[C, 0, :] = bias_row
        w[:, 1, :] = w_hh[lo:lo + H].T
        return np.ascontiguousarray(w.astype(NP_FP8))

    b1 = b_ih1 + b_hh1
    w1n29 = np.concatenate([w_ih1[2 * H:3 * H].T,
                            b_ih1[2 * H:3 * H].reshape(1, H)], axis=0)  # [29,128]
    w1n = np.zeros((64, H), np.float32)
    w1n[0:KAUG] = w1n29
    w1n[32:32 + KAUG] = w1n29

    biases = np.stack([
        b_ih2[0:H] + b_hh2[0:H],
        b_ih2[H:2 * H] + b_hh2[H:2 * H],
        b_hh1[2 * H:3 * H],
        b_hh2[2 * H:3 * H],
        b_ih2[2 * H:3 * H],
    ], axis=1).astype(np.float32)         # [H, 5]

    common = {
        "w1r": dr_weights(w_ih1, w_hh1, b1[0:H], 0),
        "w1z": dr_weights(w_ih1, w_hh1, b1[H:2 * H], H),
        "w2r": dr_weights(w_ih2, w_hh2, None, 0),
        "w2z": dr_weights(w_ih2, w_hh2, None, H),
        "w1n": np.ascontiguousarray(w1n.astype(NP_BF16)),
        "whn1": np.ascontiguousarray(w_hh1[2 * H:3 * H].T.astype(NP_BF16)),
        "win2": np.ascontiguousarray(w_ih2[2 * H:3 * H].T.astype(NP_BF16)),
        "whn2": np.ascontiguousarray(w_hh2[2 * H:3 * H].T.astype(NP_BF16)),
        "woutT": np.ascontiguousarray(w_out.T.astype(NP_BF16)),
        "biases": np.ascontiguousarray(biases),
        "bout": np.ascontiguousarray(
            np.broadcast_to(b_out.reshape(NOUT, 1), (NOUT, BF)).astype(np.float32)),
    }
    x8 = xaug.astype(NP_FP8)
    in_maps = []
    for c in range(NCORES):
        m = dict(common)
        sl = slice(c * B_CORE, (c + 1) * B_CORE)
        m["x8"] = np.ascontiguousarray(x8[:, :, sl])
        # xb packed for 2-way row tiling: [T, 64, B_CORE//2] where
        # rows 0:29 carry the even 512-col half, rows 32:61 the odd half
        xv = xaug[:, :, sl].reshape(T, KAUG, NPAIR, 2, BF)
        xb2 = np.zeros((T, 64, NPAIR, BF), np.float32)
        xb2[:, 0:KAUG] = xv[:, :, :, 0, :]
        xb2[:, 32:32 + KAUG] = xv[:, :, :, 1, :]
        m["xb"] = np.ascontiguousarray(
            xb2.reshape(T, 64, NPAIR * BF).astype(NP_BF16))
        in_maps.append(m)
    return in_maps


def kernel(**inputs):
    global LAST_RESULT
    nc = _build_bass()
    edited = _split_multi_waits(nc.to_json_bytes())
    nc.to_json_bytes = lambda: edited
    in_maps = _prep_inputs(**inputs)
    trace = bool(int(os.environ.get("BASS_TRACE", "0")))
    res = run_bass_kernel_spmd(nc, in_maps, core_ids=list(range(NCORES)),
                               trace=trace)
    LAST_RESULT = res
    outs = [r["out"] for r in res.results]          # each [NOUT, B_CORE] f32
    full = np.concatenate(outs, axis=1)             # [NOUT, N_TOTAL]
    return np.ascontiguousarray(full.T).astype(np.float32)


# revision 37
# speedup vs baseline: 1.0181x; 1.0181x over previous
"""Trainium2 Bass kernel: 2-layer GRU (H=128) over 28 timesteps + Linear head.

Reference computation (PyTorch GRUCell semantics, gates r,z,n):
    for t in 28 rows of each 28x28 image:
        h1 = relu(gru1(x_t, h1));  h2 = relu(gru2(h1, h2))
    out = h2 @ w_out.T + b_out

Sharding: pure data parallel, batch 32768 -> 8 cores x 4096.
On-chip layout: transposed [hidden=partition, batch=free]; batch 8x512,
SBUF elementwise ops pair-wide (1024 cols).

Matmul plan per cell (was 6 bf16 matmuls in the baseline, now 4):
  - r and z gates: ONE fp8 DoubleRow matmul each, fusing the input-side and
    hidden-side GEMMs (virtual K=256: plane0 = x/h1, plane1 = h_prev).
  - n gate: ghn (bf16 weights, fp8 h rhs), plus gin emitted one pipeline slot
    later with start=False, ACCUMULATING onto a psum bank pre-seeded by the
    t1 = (ghn+b)*r vector op -- so the former t2 = t1+gin add is done by the
    PE for free and tanh reads the psum directly.
fp8 h state copies are relu ops writing float8_e4m3 on DVE; the bf16 h state
is eliminated: the recurrence keeps pre-relu hp in bf16 and fuses relu into
the (h - n) subtract via scalar_tensor_tensor(max, subtract).
Validated end-to-end in numpy: rel_err ~7.6e-3 (vs 2e-2 budget).

Engine notes (measured): Pool TENSOR_SCALAR is pathological (~14.7us/1024) --
never use; Pool TENSOR_TENSOR is 2.3us/1024 (0.42 eff) -- used for the hp add
and half the t4 muls. DVE stt has no 2x mode (1.34us/1024).

Bias folding:
  - L1 r/z: ones row in the fp8 x plane; weight row 28 carries b_ih1+b_hh1.
  - L1 n: ones row in the bf16 x tile; w1n row 28 carries b_ih1n.
  - L2 r/z: ScalarE activation bias APs. L2 n: tanh bias AP (b_ih2n).
  - b_hh*n folded into the (ghn + b) * r fused scalar_tensor_tensor.
"""

import json
import os
from contextlib import ExitStack

import ml_dtypes
import numpy as np

import concourse.bass as bass
import concourse.tile as tile
from concourse import mybir
from concourse.bass_utils import run_bass_kernel_spmd

HID = 128
T = 28
C = 28
KAUG = C + 1
NCORES = 8
N_TOTAL = 32768
B_CORE = N_TOTAL // NCORES  # 4096
BF = 512                    # matmul free dim / psum bank
PW = 2 * BF                 # pair width for SBUF elementwise ops
NPAIR = B_CORE // PW        # 4
NOUT = 10

F32 = mybir.dt.float32
BF16 = mybir.dt.bfloat16
FP8 = mybir.dt.float8e4
AF = mybir.ActivationFunctionType
ALU = mybir.AluOpType
DR = mybir.MatmulPerfMode.DoubleRow

NP_FP8 = ml_dtypes.float8_e4m3
NP_BF16 = ml_dtypes.bfloat16

# stash of the last run's perf results for test harness inspection
LAST_RESULT = None


def _split_multi_waits(bir_bytes: bytes) -> bytes:
    """This walrus build rejects instructions carrying >1 sync wait
    ("Too many sync wait commands"). Split extras into standalone
    single-wait EventSemaphore instructions on the same engine, placed
    immediately before -- semantically identical blocking."""
    d = json.loads(bir_bytes)
    ctr = 0
    for fn in d["functions"]:
        for bb in fn["blocks"]:
            out = []
            for inst in bb["instructions"]:
                si = inst.get("sync_info")
                waits = (si or {}).get("on_wait") or []
                if len(waits) > 1:
                    for w in waits[:-1]:
                        ctr += 1
                        out.append({
                            "debug": inst.get("debug", 0),
                            "engine": inst.get("engine"),
                            "ins": [],
                            "outs": [],
                            "name": f"xw-{ctr}",
                            "opcode": "EventSemaphore",
                            "sync_info": {"on_update": [], "on_wait": [w]},
                        })
                    si["on_wait"] = [waits[-1]]
                out.append(inst)
            bb["instructions"] = out
    return json.dumps(d).encode()


def _build_bass() -> bass.Bass:
    nc = bass.Bass()

    x8_d = nc.dram_tensor("x8", [T, KAUG, B_CORE], FP8, kind="ExternalInput")
    # bf16 x for the L1 n-gate, pre-packed for 2-way PE row tiling:
    # rows 0-28 = x of even 512-col half, rows 32-60 = x of odd half
    xb_d = nc.dram_tensor("xb", [T, 64, B_CORE // 2], BF16, kind="ExternalInput")
    w1r_d = nc.dram_tensor("w1r", [HID, 2, HID], FP8, kind="ExternalInput")
    w1z_d = nc.dram_tensor("w1z", [HID, 2, HID], FP8, kind="ExternalInput")
    w2r_d = nc.dram_tensor("w2r", [HID, 2, HID], FP8, kind="ExternalInput")
    w2z_d = nc.dram_tensor("w2z", [HID, 2, HID], FP8, kind="ExternalInput")
    w1n_d = nc.dram_tensor("w1n", [64, HID], BF16, kind="ExternalInput")
    whn1_d = nc.dram_tensor("whn1", [HID, HID], BF16, kind="ExternalInput")
    win2_d = nc.dram_tensor("win2", [HID, HID], BF16, kind="ExternalInput")
    whn2_d = nc.dram_tensor("whn2", [HID, HID], BF16, kind="ExternalInput")
    wout_d = nc.dram_tensor("woutT", [HID, NOUT], BF16, kind="ExternalInput")
    # bias columns: 0=b2r, 1=b2z, 2=b_hh1n, 3=b_hh2n, 4=b_ih2n
    bias_d = nc.dram_tensor("biases", [HID, 5], F32, kind="ExternalInput")
    bout_d = nc.dram_tensor("bout", [NOUT, BF], F32, kind="ExternalInput")
    out_d = nc.dram_tensor("out", [NOUT, B_CORE], F32, kind="ExternalOutput")

    with ExitStack() as ctx:
        tc = ctx.enter_context(tile.TileContext(nc))

        consts = ctx.enter_context(tc.tile_pool(name="consts", bufs=1))
        prz = ctx.enter_context(tc.tile_pool(name="prz", bufs=2, space="PSUM"))
        pgi = ctx.enter_context(tc.tile_pool(name="pgi", bufs=2, space="PSUM"))
        pgh = ctx.enter_context(tc.tile_pool(name="pgh", bufs=2, space="PSUM"))
        xbp = ctx.enter_context(tc.tile_pool(name="xbp", bufs=3))
        l1p = ctx.enter_context(tc.tile_pool(name="l1p", bufs=3))
        l2p = ctx.enter_context(tc.tile_pool(name="l2p", bufs=2))
        hpp = ctx.enter_context(tc.tile_pool(name="hpp", bufs=2))
        rzp = ctx.enter_context(tc.tile_pool(name="rzp", bufs=2))
        t1sc = ctx.enter_context(tc.tile_pool(name="t1sc", bufs=1))
        npl = ctx.enter_context(tc.tile_pool(name="npl", bufs=2))
        t3p = ctx.enter_context(tc.tile_pool(name="t3p", bufs=1))
        t4p = ctx.enter_context(tc.tile_pool(name="t4p", bufs=1))
        opool = ctx.enter_context(tc.tile_pool(name="op", bufs=1))

        w1r = consts.tile([HID, 2, HID], FP8)
        nc.sync.dma_start(out=w1r, in_=w1r_d[:, :, :])
        w1z = consts.tile([HID, 2, HID], FP8)
        nc.sync.dma_start(out=w1z, in_=w1z_d[:, :, :])
        w2r = consts.tile([HID, 2, HID], FP8)
        nc.sync.dma_start(out=w2r, in_=w2r_d[:, :, :])
        w2z = consts.tile([HID, 2, HID], FP8)
        nc.sync.dma_start(out=w2z, in_=w2z_d[:, :, :])
        w1n = consts.tile([64, HID], BF16)
        nc.sync.dma_start(out=w1n, in_=w1n_d[:, :])
        whn1 = consts.tile([HID, HID], BF16)
        nc.sync.dma_start(out=whn1, in_=whn1_d[:, :])
        win2 = consts.tile([HID, HID], BF16)
        nc.sync.dma_start(out=win2, in_=win2_d[:, :])
        whn2 = consts.tile([HID, HID], BF16)
        nc.sync.dma_start(out=whn2, in_=whn2_d[:, :])
        wo = consts.tile([HID, NOUT], BF16)
        nc.sync.dma_start(out=wo, in_=wout_d[:, :])
        bs = consts.tile([HID, 5], F32)
        nc.sync.dma_start(out=bs, in_=bias_d[:, :])
        bo = consts.tile([NOUT, BF], F32)
        nc.sync.dma_start(out=bo, in_=bout_d[:, :])

        # --- prologue: state + rhs staging tiles ---
        l1t = {sp: [] for sp in range(NPAIR)}   # fp8 [x_t | h1_{t-1}] per (t, sp)
        xbt = {sp: [] for sp in range(NPAIR)}   # bf16 x_t per (t, sp)
        l2cur = {}                              # fp8 [h1_t | h2_{t-1}] per sp
        hp = {}                                 # bf16 pre-relu state per (layer, sp)

        for sp in range(NPAIR):
            for t0 in range(3):
                tl = l1p.tile([HID, 2, PW], FP8, tag=f"l1_{sp}", name=f"l1_{sp}_{t0}")
                nc.vector.memset(tl, 0.0)
                nc.sync.dma_start(out=tl[0:KAUG, 0, :],
                                  in_=x8_d[t0, :, sp * PW:(sp + 1) * PW])
                l1t[sp].append(tl)
                xt = xbp.tile([64, BF], BF16, tag=f"xb_{sp}", name=f"xb_{sp}_{t0}")
                nc.sync.dma_start(out=xt, in_=xb_d[t0, :, sp * BF:(sp + 1) * BF])
                xbt[sp].append(xt)
            l2tile = l2p.tile([HID, 2, PW], FP8, tag=f"l2_{sp}", name=f"l2i_{sp}")
            nc.gpsimd.memset(l2tile, 0.0)
            l2cur[sp] = l2tile
            h1i = hpp.tile([HID, PW], BF16, tag=f"hp1_{sp}", name=f"hp1i_{sp}")
            nc.gpsimd.memset(h1i, 0.0)
            hp[(1, sp)] = h1i
            h2i = hpp.tile([HID, PW], BF16, tag=f"hp2_{sp}", name=f"hp2i_{sp}")
            nc.gpsimd.memset(h2i, 0.0)
            hp[(2, sp)] = h2i

        pending_dma = []

        def unit(t, layer, sp):
            """One pair (t, layer, sp). PE order: ghn first (no fresh deps,
            and t1 = (ghn+b)*r is on the critical chain), then 4 DR, then the
            accumulating gin matmuls last -- padded by the 6-matmul batch so
            the in-order PE queue rarely stalls on the sigma->t1 chain."""
            if layer == 1:
                l1c = l1t[sp][t]
                rhs_of = lambda po: l1c[:, :, po * BF:(po + 1) * BF]
                ghn_of = lambda po: l1c[:, 1, po * BF:(po + 1) * BF]
                wr, wz, whn = w1r, w1z, whn1
                sig_r = sig_z = 0.0
                bhhn = bs[:, 2:3]
                tanh_bias = 0.0
            else:
                l2c = l2cur[sp]
                rhs_of = lambda po: l2c[:, :, po * BF:(po + 1) * BF]
                ghn_of = lambda po: l2c[:, 1, po * BF:(po + 1) * BF]
                wr, wz, whn = w2r, w2z, whn2
                sig_r, sig_z = bs[:, 0:1], bs[:, 1:2]
                bhhn = bs[:, 3:4]
                tanh_bias = bs[:, 4:5]

            rzt = rzp.tile([HID, 2, PW], BF16, tag=f"rz_{sp}", name=f"rz{layer}_{sp}")
            # ghn first: it has no fresh dependencies and t1 = (ghn+b)*r
            # is on the critical chain to the accumulating gin matmuls
            gi = {}
            gh = {}
            for po in range(2):
                gh[po] = pgh.tile([HID, BF], F32, tag="gh", name=f"gh_{po}")
                nc.tensor.matmul(gh[po], whn, ghn_of(po), start=True, stop=True)
            rz = {}
            for po in range(2):
                rz[po] = prz.tile([HID, 2 * BF], F32, tag="rz", name=f"rz_{po}")
                nc.tensor.matmul(rz[po][:, 0:BF], wr, rhs_of(po), start=True,
                                 stop=True, perf_mode=DR)
                nc.tensor.matmul(rz[po][:, BF:2 * BF], wz, rhs_of(po),
                                 start=True, stop=True, perf_mode=DR)
            for po in range(2):
                # sigma_r first, then the t1 seed (which only needs r), then
                # sigma_z -- t1 is on the critical chain to the gin matmuls
                nc.scalar.activation(rzt[:, 0, po * BF:(po + 1) * BF],
                                     rz[po][:, 0:BF], AF.Sigmoid, bias=sig_r)
                # seed the gin psum bank with t1 = (ghn + b_hhn) * r;
                # the gin matmul then accumulates on top (start=False)
                gi[po] = pgi.tile([HID, BF], F32, tag="gi", name=f"gi_{po}")
                nc.vector.scalar_tensor_tensor(
                    gi[po], gh[po], bhhn,
                    rzt[:, 0, po * BF:(po + 1) * BF], op0=ALU.add, op1=ALU.mult)
                nc.scalar.activation(rzt[:, 1, po * BF:(po + 1) * BF],
                                     rz[po][:, BF:2 * BF], AF.Sigmoid,
                                     bias=sig_z)
            # h1 fp8 copies deferred one slot: by now the producing relu is
            # long done, so the issue never head-of-line-blocks the Sync DMA
            # queue (which cost ~10us/copy when issued eagerly).
            while pending_dma:
                dst, src = pending_dma.pop()
                nc.sync.dma_start(out=dst, in_=src)
            if layer == 1:
                # 2-way row tiling: both 512-col halves' gins run concurrently
                # in PE row groups 0 and 32 (K=29 each)
                xt = xbt[sp][t]
                nc.tensor.matmul(gi[0], w1n[0:KAUG, :], xt[0:KAUG, :],
                                 start=False, stop=True, skip_group_check=True,
                                 tile_position=(0, 0))
                nc.tensor.matmul(gi[1], w1n[32:32 + KAUG, :],
                                 xt[32:32 + KAUG, :],
                                 start=False, stop=True, skip_group_check=True,
                                 tile_position=(32, 0))
            else:
                for po in range(2):
                    nc.tensor.matmul(gi[po], win2,
                                     l2cur[sp][:, 0, po * BF:(po + 1) * BF],
                                     start=False, stop=True,
                                     skip_group_check=True)
            nn = npl.tile([HID, PW], BF16, tag=f"n_{sp}", name=f"n{layer}_{sp}")
            for po in range(2):
                nc.scalar.activation(nn[:, po * BF:(po + 1) * BF], gi[po],
                                     AF.Tanh, bias=tanh_bias)

            def tail():
                # State update + relus, deferred one slot so the NEXT unit's
                # t1 seed precedes these ops on the DVE queue -- breaking the
                # t1 -> tail(prev) -> tanh -> gin -> t1 circular queue wait.
                hp_key = (layer, sp)
                # t3 = relu(hp_prev) - n  (fused relu via max-then-subtract)
                t3 = t3p.tile([HID, PW], BF16, tag=f"t3_{sp}", name="t3")
                nc.vector.scalar_tensor_tensor(t3, hp[hp_key], 0.0, nn,
                                               op0=ALU.max, op1=ALU.subtract)
                t4 = t4p.tile([HID, PW], BF16, tag=f"t4_{sp}", name="t4")
                t4_eng = nc.vector if sp % 2 == 0 else nc.gpsimd
                t4_eng.tensor_tensor(t4, rzt[:, 1, :], t3, op=ALU.mult)
                hpn = hpp.tile([HID, PW], BF16, tag=f"hp{layer}_{sp}",
                               name=f"hp{layer}_{sp}_n")
                nc.gpsimd.tensor_tensor(hpn, nn, t4, op=ALU.add)
                hp[hp_key] = hpn

                if layer == 1:
                    # fp8 relu'd h1 -> L2 rhs plane0 (same t)
                    nc.vector.tensor_scalar_max(l2cur[sp][:, 0, :], hpn, 0.0)
                    # copy h1 fp8 -> next L1 rhs plane1 (issue deferred)
                    if t + 1 < T:
                        pending_dma.append((l1t[sp][t + 1][:, 1, :],
                                            l2cur[sp][:, 0, :]))
                    # stage t+3 inputs (prologue covered 0..2)
                    if t + 3 < T:
                        tl = l1p.tile([HID, 2, PW], FP8, tag=f"l1_{sp}",
                                      name=f"l1_{sp}_{t + 3}")
                        nc.sync.dma_start(
                            out=tl[0:KAUG, 0, :],
                            in_=x8_d[t + 3, :, sp * PW:(sp + 1) * PW])
                        l1t[sp].append(tl)
                        xt = xbp.tile([64, BF], BF16, tag=f"xb_{sp}",
                                      name=f"xb_{sp}_{t + 3}")
                        nc.sync.dma_start(
                            out=xt, in_=xb_d[t + 3, :, sp * BF:(sp + 1) * BF])
                        xbt[sp].append(xt)
                else:
                    if t + 1 < T:
                        l2n = l2p.tile([HID, 2, PW], FP8, tag=f"l2_{sp}",
                                       name=f"l2_{sp}_{t + 1}")
                        nc.vector.tensor_scalar_max(l2n[:, 1, :], hpn, 0.0)
                        l2cur[sp] = l2n

            return tail

        for t in range(T):
            for layer in (1, 2):
                for sp in range(NPAIR):
                    unit(t, layer, sp)()

        # ---- head: out = relu(hp2) @ wout + bout ----
        ob = opool.tile([NOUT, B_CORE], F32, tag="ob")
        for sp in range(NPAIR):
            h2f = npl.tile([HID, PW], BF16, tag=f"n_{sp}", name=f"h2f_{sp}")
            nc.vector.tensor_scalar_max(h2f, hp[(2, sp)], 0.0)
            for po in range(2):
                pout = pgh.tile([NOUT, BF], F32, tag="gh")
                nc.tensor.matmul(pout, wo, h2f[:, po * BF:(po + 1) * BF],
                                 start=True, stop=True)
                s = sp * 2 + po
                nc.vector.tensor_tensor(ob[:, s * BF:(s + 1) * BF], pout, bo,
                                        op=ALU.add)
        nc.scalar.dma_start(out=out_d[:, :], in_=ob)

    return nc


def _prep_inputs(x, w_ih1, w_hh1, b_ih1, b_hh1, w_ih2, w_hh2, b_ih2, b_hh2,
                 w_out, b_out):
    """Host-side reshape/transpose/cast + per-core sharding."""
    n = N_TOTAL
    H = HID
    xs = np.asarray(x, np.float32).reshape(n, T, C)       # channel dim is 1
    xt = np.transpose(xs, (1, 2, 0))                      # [T, C, n]
    xaug = np.concatenate(
        [xt, np.ones((T, 1, n), np.float32)], axis=1)     # [T, KAUG, n]

    w_ih1 = np.asarray(w_ih1, np.float32)
    w_hh1 = np.asarray(w_hh1, np.float32)
    b_ih1 = np.asarray(b_ih1, np.float32)
    b_hh1 = np.asarray(b_hh1, np.float32)
    w_ih2 = np.asarray(w_ih2, np.float32)
    w_hh2 = np.asarray(w_hh2, np.float32)
    b_ih2 = np.asarray(b_ih2, np.float32)
    b_hh2 = np.asarray(b_hh2, np.float32)
    w_out = np.asarray(w_out, np.float32)
    b_out = np.asarray(b_out, np.float32)

    def dr_weights(w_ih, w_hh, bias_row, lo):
        """fp8 DoubleRow lhsT [128, 2, 128] for one gate (rows lo:lo+H)."""
        w = np.zeros((HID, 2, HID), np.float32)
        kin = w_ih.shape[1]
        w[0:kin, 0, :] = w_ih[lo:lo + H].T
        if bias_row is not None:
            w[C, 0, :] = bias_row
        w[:, 1, :] = w_hh[lo:lo + H].T
        return np.ascontiguousarray(w.astype(NP_FP8))

    b1 = b_ih1 + b_hh1
    w1n29 = np.concatenate([w_ih1[2 * H:3 * H].T,
                            b_ih1[2 * H:3 * H].reshape(1, H)], axis=0)  # [29,128]
    w1n = np.zeros((64, H), np.float32)
    w1n[0:KAUG] = w1n29
    w1n[32:32 + KAUG] = w1n29

    biases = np.stack([
        b_ih2[0:H] + b_hh2[0:H],
        b_ih2[H:2 * H] + b_hh2[H:2 * H],
        b_hh1[2 * H:3 * H],
        b_hh2[2 * H:3 * H],
        b_ih2[2 * H:3 * H],
    ], axis=1).astype(np.float32)         # [H, 5]

    common = {
        "w1r": dr_weights(w_ih1, w_hh1, b1[0:H], 0),
        "w1z": dr_weights(w_ih1, w_hh1, b1[H:2 * H], H),
        "w2r": dr_weights(w_ih2, w_hh2, None, 0),
        "w2z": dr_weights(w_ih2, w_hh2, None, H),
        "w1n": np.ascontiguousarray(w1n.astype(NP_BF16)),
        "whn1": np.ascontiguousarray(w_hh1[2 * H:3 * H].T.astype(NP_BF16)),
        "win2": np.ascontiguousarray(w_ih2[2 * H:3 * H].T.astype(NP_BF16)),
        "whn2": np.ascontiguousarray(w_hh2[2 * H:3 * H].T.astype(NP_BF16)),
        "woutT": np.ascontiguousarray(w_out.T.astype(NP_BF16)),
        "biases": np.ascontiguousarray(biases),
        "bout": np.ascontiguousarray(
            np.broadcast_to(b_out.reshape(NOUT, 1), (NOUT, BF)).astype(np.float32)),
    }
    x8 = xaug.astype(NP_FP8)
    in_maps = []
    for c in range(NCORES):
        m = dict(common)
        sl = slice(c * B_CORE, (c + 1) * B_CORE)
        m["x8"] = np.ascontiguousarray(x8[:, :, sl])
        # xb packed for 2-way row tiling: [T, 64, B_CORE//2] where
        # rows 0:29 carry the even 512-col half, rows 32:61 the odd half
        xv = xaug[:, :, sl].reshape(T, KAUG, NPAIR, 2, BF)
        xb2 = np.zeros((T, 64, NPAIR, BF), np.float32)
        xb2[:, 0:KAUG] = xv[:, :, :, 0, :]
        xb2[:, 32:32 + KAUG] = xv[:, :, :, 1, :]
        m["xb"] = np.ascontiguousarray(
            xb2.reshape(T, 64, NPAIR * BF).astype(NP_BF16))
        in_maps.append(m)
    return in_maps


def kernel(**inputs):
    global LAST_RESULT
    nc = _build_bass()
    edited = _split_multi_waits(nc.to_json_bytes())
    nc.to_json_bytes = lambda: edited
    in_maps = _prep_inputs(**inputs)
    trace = bool(int(os.environ.get("BASS_TRACE", "0")))
    res = run_bass_kernel_spmd(nc, in_maps, core_ids=list(range(NCORES)),
                               trace=trace)
    LAST_RESULT = res
    outs = [r["out"] for r in res.results]          # each [NOUT, B_CORE] f32
    full = np.concatenate(outs, axis=1)             # [NOUT, N_TOTAL]
    return np.ascontiguousarray(full.T).astype(np.float32)
